# revision 6
# baseline (speedup 1.0000x reference)
"""Trainium2 Bass kernel for nn_DiffusionNetwork (30-step diffusion sampling).

Algorithm (exact algebraic restructuring of the reference):
  The MLP input ``cond = z + time_embed[t]`` is independent of the scanned
  ``action``, so:
    1. u = z @ W1 is computed ONCE (the t-loop adds only a rank-1 shift):
       h_t = gelu(u + v_t)  with  v_t = time_embed[t] @ W1 + b1  (host precomp)
    2. The sequential scan is linear in (pred_t, noise_t), so it collapses to
       a weighted sum with host-precomputed scalar weights:
       action = w_init*init + sum_t wp[t]*(h_t @ W2 + b2) + sum_t wn[t]*noise_t
  This cuts FLOPs 16x vs the naive 30 full MLP passes and removes every
  sequential dependency.

Sharding: data-parallel over batch (B=16384 -> 2048/core on 8 cores).
Per-core layouts are transposed host-side so the contraction dim lands on
SBUF partitions: u is kept resident in SBUF as uT [d, b] (16 tiles of
[128, 2048] f32), gelu runs on ScalarE with v_t as the per-partition bias,
and the pred matmuls use W2 as the stationary operand (out = predT
[64 a, 512 b] in PSUM, accumulated over the 16 d-tiles).

Matmul operands are fp16: same 10-bit-mantissa input rounding as tf32
(float32r) but at full 1 cycle/row PE rate with prefetchable weight loads
(fp32/float32r "HIGH"-mode matmuls measured ~2x slower with serialized
LDWEIGHTS). Accumulation is always fp32 in PSUM. zT is fully SBUF-resident
in fp16 so phase 1 loads each W1 weight tile once and streams all four
512-wide b-chunks through it.
"""

import sys

import numpy as np

try:
    import concourse  # noqa: F401
except ImportError:
    sys.path.insert(0, "/opt/trn_rl_repo")

import concourse.bass as bass
import concourse.tile as tile
from concourse import bacc, mybir
from concourse import bass_utils

F32 = mybir.dt.float32
F16 = mybir.dt.float16

STEPS = 30
B, D, A = 16384, 2048, 64
NCORES = 8
BL = B // NCORES          # 2048 batch rows per core
KT = D // 128             # 16 contraction tiles
MT = D // 128             # 16 output-row tiles of u
NB = 512                  # moving-dim chunk (one PSUM bank of fp32)
QT = BL // NB             # 4 b-chunks per core


def _schedule_weights():
    """Host constant-folding of the diffusion schedule + scan collapse."""
    t = np.linspace(0.0, STEPS, STEPS + 1) / STEPS
    ab = np.cos((t + 0.008) / 1.008 * np.pi / 2) ** 2
    ab = ab / ab[0]
    beta = np.clip(1.0 - ab[1:] / ab[:-1], 0.0, 0.999)
    alpha = 1.0 - beta
    alpha_bar = np.cumprod(alpha)
    c1 = (1.0 - alpha) / np.sqrt(1.0 - alpha_bar)
    c2 = 1.0 / np.sqrt(alpha)
    c3 = np.sqrt(beta)
    c3[0] = 0.0
    w_init = 1.0
    wp = np.zeros(STEPS)
    wn = np.zeros(STEPS)
    for tt in range(STEPS - 1, -1, -1):  # scan order
        w_init *= c2[tt]
        wp *= c2[tt]
        wn *= c2[tt]
        wp[tt] = -c1[tt] * c2[tt]
        wn[tt] = c3[tt]
    return float(w_init), wp, wn


_W_INIT, _WP, _WN = _schedule_weights()

_PROGRAM = None  # cached compiled Bass program


def _build_program():
    nc = bacc.Bacc("TRN2", target_bir_lowering=False, debug=False,
                   num_devices=NCORES)

    zT_d = nc.dram_tensor("zT", [D, BL], F16, kind="ExternalInput")
    w1t_d = nc.dram_tensor("w1t", [MT, D, 128], F16, kind="ExternalInput")
    w2_d = nc.dram_tensor("w2", [D, A], F16, kind="ExternalInput")
    vT_d = nc.dram_tensor("vT", [D, STEPS], F32, kind="ExternalInput")
    initT_d = nc.dram_tensor("initT", [A, BL], F32, kind="ExternalInput")
    noiseT_d = nc.dram_tensor("noiseT", [STEPS, A, BL], F32, kind="ExternalInput")
    b2s_d = nc.dram_tensor("b2s", [A, 1], F32, kind="ExternalInput")
    outT_d = nc.dram_tensor("outT", [A, BL], F32, kind="ExternalOutput")

    GELU = mybir.ActivationFunctionType.Gelu
    MUL = mybir.AluOpType.mult
    ADD = mybir.AluOpType.add

    with tile.TileContext(nc) as tc:
        with tc.tile_pool(name="u", bufs=1) as u_pool, \
             tc.tile_pool(name="zp", bufs=1) as z_pool, \
             tc.tile_pool(name="w2p", bufs=1) as w2_pool, \
             tc.tile_pool(name="vtp", bufs=1) as vt_pool, \
             tc.tile_pool(name="hp", bufs=8) as h_pool, \
             tc.tile_pool(name="accp", bufs=1) as acc_pool:
            u = [u_pool.tile([128, BL], F16, tag=f"u{m}", name=f"u{m}")
                 for m in range(MT)]
            zk = [z_pool.tile([128, BL], F16, tag=f"z{k}", name=f"zk{k}")
                  for k in range(KT)]
            for k in range(KT):
                nc.sync.dma_start(zk[k][:],
                                  zT_d.ap()[k * 128:(k + 1) * 128, :])
            w2 = [w2_pool.tile([128, A], F16, tag=f"w2{m}", name=f"w2{m}")
                  for m in range(MT)]
            vt = [vt_pool.tile([128, STEPS], F32, tag=f"vt{m}", name=f"vt{m}")
                  for m in range(MT)]
            for m in range(MT):
                nc.sync.dma_start(w2[m][:], w2_d.ap()[m * 128:(m + 1) * 128, :])
                nc.sync.dma_start(vt[m][:], vT_d.ap()[m * 128:(m + 1) * 128, :])
            b2s = acc_pool.tile([A, 1], F32, name="b2s")
            nc.sync.dma_start(b2s[:], b2s_d.ap()[:])
            # noise/init weighted sum: host pre-scales by wn[t]/w_init, device
            # accumulates with GPSIMD software-DGE DMA adds (keeps DVE free).
            acc_nz = acc_pool.tile([A, BL], F32, name="acc_nz")
            nc.sync.dma_start(acc_nz[:], initT_d.ap()[:])
            for t in range(STEPS):
                if _WN[t] == 0.0:
                    continue
                nc.gpsimd.dma_start(acc_nz[:], noiseT_d.ap()[t],
                                    accum_op=mybir.AluOpType.add)
            acc = acc_pool.tile([A, BL], F32, name="acc")

            # ---- Phase 1 + step 0 interleaved, then steps 1..29 ----
            # zT fully SBUF-resident in fp16; each W1 k-tile loaded once and
            # reused across all 4 b-chunks. Step 0's gelu+matmuls are emitted
            # inside the phase-1 m-loop so ACT/PE overlap the u build.
            with tc.tile_pool(name="ps2", bufs=1, space="PSUM") as ps2:
                pp = [ps2.tile([A, NB], F32, tag=f"pp{q}", name=f"pp{q}")
                      for q in range(QT)]
                with tc.tile_pool(name="w1p", bufs=8) as w1_pool, \
                     tc.tile_pool(name="ps1", bufs=1, space="PSUM") as ps1:
                    for m in range(MT):
                        ps = [ps1.tile([128, NB], F32, tag=f"pa{q}",
                                       name=f"ps{q}")
                              for q in range(QT)]
                        for k in range(KT):
                            w1 = w1_pool.tile([128, 128], F16, tag="w1",
                                              name="w1")
                            nc.sync.dma_start(
                                w1[:], w1t_d.ap()[m, k * 128:(k + 1) * 128, :])
                            for q in range(QT):
                                nc.tensor.matmul(
                                    ps[q][:], w1[:],
                                    zk[k][:, q * NB:(q + 1) * NB],
                                    start=(k == 0), stop=(k == KT - 1))
                        for q in range(QT):
                            nc.vector.tensor_copy(u[m][:, q * NB:(q + 1) * NB],
                                                  ps[q][:])
                        # step 0 for this m-tile rides along with phase 1
                        h = h_pool.tile([128, BL], F16, tag="h", name="h")
                        nc.scalar.activation(h[:], u[m][:], GELU,
                                             bias=vt[m][:, 0:1], scale=1.0)
                        for q in range(QT):
                            nc.tensor.matmul(
                                pp[q][:], w2[m][:],
                                h[:, q * NB:(q + 1) * NB],
                                start=(m == 0), stop=(m == MT - 1))
                for q in range(QT):
                    nc.vector.tensor_scalar_mul(
                        acc[:, q * NB:(q + 1) * NB], pp[q][:], float(_WP[0]))

                for t in range(1, STEPS):
                    pp = [ps2.tile([A, NB], F32, tag=f"pp{q}", name=f"pp{q}")
                          for q in range(QT)]
                    for m in range(MT):
                        h = h_pool.tile([128, BL], F16, tag="h", name="h")
                        nc.scalar.activation(h[:], u[m][:], GELU,
                                             bias=vt[m][:, t:t + 1], scale=1.0)
                        for q in range(QT):
                            nc.tensor.matmul(
                                pp[q][:], w2[m][:],
                                h[:, q * NB:(q + 1) * NB],
                                start=(m == 0), stop=(m == MT - 1))
                    for q in range(QT):
                        nc.vector.scalar_tensor_tensor(
                            acc[:, q * NB:(q + 1) * NB], pp[q][:],
                            float(_WP[t]), acc[:, q * NB:(q + 1) * NB],
                            op0=MUL, op1=ADD)

                # out = pred_acc + noise_acc + (sum_t wp[t]) * b2
                nc.vector.tensor_add(acc[:], acc[:], acc_nz[:])
                nc.vector.tensor_scalar_add(acc[:], acc[:], b2s[:, 0:1])
                nc.sync.dma_start(outT_d.ap()[:], acc[:])

    nc.compile()
    return nc


def _get_program():
    global _PROGRAM
    if _PROGRAM is None:
        _PROGRAM = _build_program()
    return _PROGRAM


def kernel(z, time_embed, W1, b1, W2, b2, init_noise, step_noise,
           _bass_results=None):
    z = np.asarray(z, dtype=np.float32)
    W1 = np.asarray(W1, dtype=np.float32)
    W2 = np.asarray(W2, dtype=np.float32)

    # host precompute: v_t = time_embed @ W1 + b1 (0.1% of total FLOPs)
    V = (time_embed.astype(np.float64) @ W1.astype(np.float64)
         + b1.astype(np.float64))
    vT = np.ascontiguousarray(V.T, dtype=np.float32)            # [D, STEPS]
    b2s = (np.float64(_WP.sum()) * b2.astype(np.float64)).astype(
        np.float32).reshape(A, 1)

    w1t = np.ascontiguousarray(
        W1.reshape(D, MT, 128).transpose(1, 0, 2)).astype(np.float16)
    w2f = W2.astype(np.float16)

    zT = z.T.astype(np.float16)                                 # [D, B]
    nc = _get_program()

    in_maps = []
    for c in range(NCORES):
        bsl = slice(c * BL, (c + 1) * BL)
        in_maps.append({
            "zT": np.ascontiguousarray(zT[:, bsl]),
            "w1t": w1t,
            "w2": w2f,
            "vT": vT,
            "initT": np.ascontiguousarray(
                (_W_INIT * init_noise[bsl].astype(np.float64)).T
                ).astype(np.float32),
            "noiseT": np.ascontiguousarray(
                (_WN[:, None, None]
                 * step_noise[:, bsl, :].astype(np.float64)
                 ).transpose(0, 2, 1)).astype(np.float32),
            "b2s": b2s,
        })

    res = bass_utils.run_bass_kernel_spmd(
        nc, in_maps, core_ids=list(range(NCORES)))
    if _bass_results is not None:
        _bass_results.append(res)

    out = np.empty((B, A), dtype=np.float32)
    for c in range(NCORES):
        out[c * BL:(c + 1) * BL] = res.results[c]["outT"].T
    return out
